# revision 1
# baseline (speedup 1.0000x reference)
"""HGNN layer on 8 Trainium2 NeuronCores (Bass/Tile).

Reference computation:
    x1 = x @ W1                                    [N, F]
    w = softmax(where(seq > 0, 1, -9e15))          uniform over valid slots
    edge = relu(sum_l w[e,l] * x1[seq[e,l]])       [E, F]
    e1 = edge @ W2                                 [E, F]
    uw = softmax(where(useq > 0, 1, -9e15))
    node = sum_l uw[n,l] * e1[useq[n,l]]           [N, F]

Strategy (8-way SPMD):
  - shard nodes/edges by rows; W1/W2 replicated
  - x1 shard computed on each core -> AllGather to a full x1 table (+zero row)
  - stage 1: batched indirect-DMA row gathers (128 rows/instr) from x1 table,
    in-place halving-tree reduce, uniform-weight correction via per-row count,
    relu, @W2 (PE transpose + matmul) -> e1 shard -> AllGather (+zero row)
  - stage 2: same gather+reduce from e1 table -> node shard -> output
  - padding slots (id 0) are remapped host-side to the zero row; counts are
    computed on device; the all-padding case adds row0 of the table exactly
    as the reference softmax does.
"""

import sys

sys.path.insert(0, "/opt/trn_rl_repo")

import numpy as np

N = 50000
E = 25000
F = 256
L = 32
P = 128
NC_COUNT = 8
NSH = N // NC_COUNT        # 6250 nodes per core
ESH = E // NC_COUNT        # 3125 edges per core
NSH_PAD = 6272             # 49 tiles
ESH_PAD = 3200             # 25 tiles
N_TILES_NODE = NSH_PAD // P
N_TILES_EDGE = ESH_PAD // P


def build_program():
    from concourse import bacc, bass, mybir, tile  # noqa: F401
    from concourse.masks import make_identity

    fp32 = mybir.dt.float32
    i32 = mybir.dt.int32

    nc = bacc.Bacc("TRN2", target_bir_lowering=False, debug=False,
                   num_devices=NC_COUNT)

    xts = nc.dram_tensor("xts", [F, NSH_PAD], fp32, kind="ExternalInput").ap()
    w1 = nc.dram_tensor("w1", [F, F], fp32, kind="ExternalInput").ap()
    w2 = nc.dram_tensor("w2", [F, F], fp32, kind="ExternalInput").ap()
    seqp = nc.dram_tensor("seqp", [ESH_PAD, L], i32, kind="ExternalInput").ap()
    useqp = nc.dram_tensor("useqp", [NSH_PAD, L], i32, kind="ExternalInput").ap()
    out = nc.dram_tensor("out", [NSH_PAD, F], fp32, kind="ExternalOutput").ap()

    AL = mybir.AluOpType

    with tile.TileContext(nc) as tc:
        with (
            tc.tile_pool(name="cst", bufs=1) as cst,
            tc.tile_pool(name="lhs", bufs=3) as lhsp,
            tc.tile_pool(name="sb", bufs=3) as sbp,
            tc.tile_pool(name="idx", bufs=8) as idxp,
            tc.tile_pool(name="gb", bufs=3) as gbp,
            tc.tile_pool(name="st", bufs=4) as stp,
            tc.tile_pool(name="ps", bufs=2, space="PSUM") as psp,
            tc.tile_pool(name="pst", bufs=2, space="PSUM") as pstp,
            tc.tile_pool(name="dram", bufs=1, space="DRAM") as dram,
        ):
            # ---------- constants ----------
            ident = cst.tile([P, P], fp32)
            make_identity(nc, ident[:])
            w1sb = [cst.tile([P, F], fp32, name=f"w1k{k}") for k in range(2)]
            w2sb = [cst.tile([P, F], fp32, name=f"w2k{k}") for k in range(2)]
            for k in range(2):
                nc.sync.dma_start(out=w1sb[k][:], in_=w1[k * P:(k + 1) * P, :])
                nc.sync.dma_start(out=w2sb[k][:], in_=w2[k * P:(k + 1) * P, :])
            zrow = cst.tile([1, F], fp32)
            nc.vector.memset(zrow[:], 0.0)
            ones1p = cst.tile([1, P], fp32)
            nc.vector.memset(ones1p[:], 1.0)

            # ---------- DRAM scratch ----------
            # each core's shard carries a trailing zero row so the AllGather
            # output (the gather table) contains zero rows without a second
            # writer on the Shared tensor
            x1loc = dram.tile([NSH + 1, F], fp32)
            x1tab = dram.tile([NC_COUNT * (NSH + 1), F], fp32, addr_space="Shared")
            e1loc = dram.tile([ESH + 1, F], fp32)
            e1tab = dram.tile([NC_COUNT * (ESH + 1), F], fp32, addr_space="Shared")

            # ---------- stage 0: x1 shard = x_shard @ W1 ----------
            with nc.named_scope("stage0"):
                for t in range(N_TILES_NODE):
                    pr = min(P, NSH - t * P)  # rows to store
                    ps0 = psp.tile([P, F], fp32, tag="mm")
                    for kc in range(2):
                        lt = lhsp.tile([P, P], fp32, tag="lhs")
                        nc.sync.dma_start(
                            out=lt[:],
                            in_=xts[kc * P:(kc + 1) * P, t * P:(t + 1) * P],
                        )
                        nc.tensor.matmul(ps0[:], lt[:], w1sb[kc][:],
                                         start=(kc == 0), stop=(kc == 1))
                    if pr > 0:
                        x1sb = sbp.tile([P, F], fp32, tag="row")
                        nc.vector.tensor_copy(out=x1sb[:pr, :], in_=ps0[:pr, :])
                        nc.sync.dma_start(out=x1loc[t * P:t * P + pr, :],
                                          in_=x1sb[:pr, :])
                nc.sync.dma_start(out=x1loc[NSH:NSH + 1, :], in_=zrow[:])
                nc.gpsimd.collective_compute(
                    "AllGather", AL.bypass,
                    replica_groups=[list(range(NC_COUNT))],
                    ins=[x1loc.opt()], outs=[x1tab.opt()],
                )

            x1row0 = cst.tile([1, F], fp32)
            nc.sync.dma_start(out=x1row0[:], in_=x1tab[0:1, :])
            x1row0b = cst.tile([P, F], fp32)
            psb = psp.tile([P, F], fp32, tag="mm")
            nc.tensor.matmul(psb[:], ones1p[:], x1row0[:], start=True, stop=True)
            nc.vector.tensor_copy(out=x1row0b[:], in_=psb[:])

            def gather_reduce(idx_dram, t, table, zval, row0):
                """One 128-row tile: gather 32 rows/slot, tree-reduce, correct.
                Returns SBUF [P, F] f32 aggregate (uniform-softmax output)."""
                idx_t = idxp.tile([P, L], i32, tag="idx")
                nc.scalar.dma_start(out=idx_t[:], in_=idx_dram[t * P:(t + 1) * P, :])
                g = gbp.tile([P, L, F], fp32, tag="gb")
                for l in range(L):
                    nc.gpsimd.indirect_dma_start(
                        out=g[:, l, :],
                        out_offset=None,
                        in_=table[:],
                        in_offset=bass.IndirectOffsetOnAxis(
                            ap=idx_t[:, l:l + 1], axis=0),
                    )
                # in-place halving tree over the 32 slots
                h = L
                while h > 1:
                    h //= 2
                    nc.vector.tensor_tensor(
                        out=g[:, 0:h, :], in0=g[:, 0:h, :], in1=g[:, h:2 * h, :],
                        op=AL.add,
                    )
                # counts: slots remapped to zero row == nrows
                idx_f = stp.tile([P, L], fp32, tag="idxf")
                nc.vector.tensor_copy(out=idx_f[:], in_=idx_t[:])
                eqz = stp.tile([P, L], fp32, tag="eqz")
                nc.vector.tensor_scalar(
                    out=eqz[:], in0=idx_f[:], scalar1=float(zval), scalar2=None,
                    op0=AL.is_equal)
                n0 = stp.tile([P, 1], fp32, tag="n0")
                nc.vector.tensor_reduce(
                    out=n0[:], in_=eqz[:], axis=mybir.AxisListType.X, op=AL.add)
                cnt = stp.tile([P, 1], fp32, tag="cnt")
                nc.vector.tensor_scalar(
                    out=cnt[:], in0=n0[:], scalar1=-1.0, scalar2=float(L),
                    op0=AL.mult, op1=AL.add)
                cmax = stp.tile([P, 1], fp32, tag="cmax")
                nc.vector.tensor_scalar(
                    out=cmax[:], in0=cnt[:], scalar1=1.0, scalar2=None,
                    op0=AL.max)
                rec = stp.tile([P, 1], fp32, tag="rec")
                nc.vector.reciprocal(out=rec[:], in_=cmax[:])
                emp = stp.tile([P, 1], fp32, tag="emp")
                nc.vector.tensor_scalar(
                    out=emp[:], in0=cnt[:], scalar1=0.0, scalar2=None,
                    op0=AL.is_equal)
                agg = sbp.tile([P, F], fp32, tag="agg")
                nc.vector.tensor_scalar(
                    out=agg[:], in0=g[:, 0, :], scalar1=rec[:], scalar2=None,
                    op0=AL.mult)
                tmp = sbp.tile([P, F], fp32, tag="tmp")
                nc.vector.tensor_scalar(
                    out=tmp[:], in0=row0[:], scalar1=emp[:],
                    scalar2=None, op0=AL.mult)
                nc.vector.tensor_tensor(
                    out=agg[:], in0=agg[:], in1=tmp[:], op=AL.add)
                return agg

            # ---------- stage 1: edges ----------
            with nc.named_scope("stage1"):
                for t in range(N_TILES_EDGE):
                    agg = gather_reduce(seqp, t, x1tab, NSH, x1row0b)
                    edge = sbp.tile([P, F], fp32, tag="edge")
                    nc.vector.tensor_scalar(
                        out=edge[:], in0=agg[:], scalar1=0.0, scalar2=None,
                        op0=AL.max)  # relu
                    # e1 = edge @ W2 : transpose edge tile then matmul
                    ps2 = psp.tile([P, F], fp32, tag="mm")
                    for kc in range(2):
                        pst = pstp.tile([P, P], fp32, tag="tr")
                        nc.tensor.transpose(
                            out=pst[:], in_=edge[:, kc * P:(kc + 1) * P],
                            identity=ident[:])
                        edgeT = sbp.tile([P, P], fp32, tag="edgeT")
                        nc.vector.tensor_copy(out=edgeT[:], in_=pst[:])
                        nc.tensor.matmul(ps2[:], edgeT[:], w2sb[kc][:],
                                         start=(kc == 0), stop=(kc == 1))
                    pr = min(P, ESH - t * P)
                    if pr > 0:
                        e1sb = sbp.tile([P, F], fp32, tag="row")
                        nc.vector.tensor_copy(out=e1sb[:pr, :], in_=ps2[:pr, :])
                        nc.sync.dma_start(out=e1loc[t * P:t * P + pr, :],
                                          in_=e1sb[:pr, :])
                nc.sync.dma_start(out=e1loc[ESH:ESH + 1, :], in_=zrow[:])
                nc.gpsimd.collective_compute(
                    "AllGather", AL.bypass,
                    replica_groups=[list(range(NC_COUNT))],
                    ins=[e1loc.opt()], outs=[e1tab.opt()],
                )

            e1row0 = cst.tile([1, F], fp32)
            nc.sync.dma_start(out=e1row0[:], in_=e1tab[0:1, :])
            e1row0b = cst.tile([P, F], fp32)
            psb2 = psp.tile([P, F], fp32, tag="mm")
            nc.tensor.matmul(psb2[:], ones1p[:], e1row0[:], start=True, stop=True)
            nc.vector.tensor_copy(out=e1row0b[:], in_=psb2[:])

            # ---------- stage 2: nodes ----------
            with nc.named_scope("stage2"):
                for t in range(N_TILES_NODE):
                    agg = gather_reduce(useqp, t, e1tab, ESH, e1row0b)
                    nc.sync.dma_start(out=out[t * P:(t + 1) * P, :], in_=agg[:])

    nc.compile()
    return nc


def make_in_maps(x, seq, useq, W1, W2):
    x = np.asarray(x, dtype=np.float32)
    W1 = np.asarray(W1, dtype=np.float32)
    W2 = np.asarray(W2, dtype=np.float32)
    seq = np.asarray(seq)
    useq = np.asarray(useq)

    # map global ids into the AllGather table layout (shard + its zero row);
    # padding slots (id 0) -> the zero row at position NSH/ESH of shard 0
    seq_m = np.where(seq > 0,
                     (seq // NSH) * (NSH + 1) + seq % NSH,
                     NSH).astype(np.int32)
    useq_m = np.where(useq > 0,
                      (useq // ESH) * (ESH + 1) + useq % ESH,
                      ESH).astype(np.int32)

    xt = np.ascontiguousarray(x.T)  # [F, N]

    in_maps = []
    for c in range(NC_COUNT):
        xts = np.zeros((F, NSH_PAD), np.float32)
        xts[:, :NSH] = xt[:, c * NSH:(c + 1) * NSH]
        seqp = np.full((ESH_PAD, L), NSH, np.int32)
        seqp[:ESH] = seq_m[c * ESH:(c + 1) * ESH]
        useqp = np.full((NSH_PAD, L), ESH, np.int32)
        useqp[:NSH] = useq_m[c * NSH:(c + 1) * NSH]
        in_maps.append({
            "xts": xts,
            "w1": W1,
            "w2": W2,
            "seqp": seqp,
            "useqp": useqp,
        })
    return in_maps


def kernel(x, seq, useq, W1, W2):
    from concourse.bass_utils import run_bass_kernel_spmd

    in_maps = make_in_maps(x, seq, useq, W1, W2)
    nc = build_program()
    res = run_bass_kernel_spmd(nc, in_maps, core_ids=list(range(NC_COUNT)),
                               trace=False)
    parts = [res.results[c]["out"][:NSH] for c in range(NC_COUNT)]
    return np.concatenate(parts, axis=0)

